# revision 1
# baseline (speedup 1.0000x reference)
"""Trainium2 Bass kernel for the DCE (dynamic contrast-enhanced) 2CXM signal model.

Math (matches reference.py exactly, restructured for TRN2):

  For each pixel: theta_m = 2/(s+d), theta_p = (s+d)/(2*Tc*Te) with
  s = T+Te, d = sqrt(s^2 - 4*Tc*Te).  em[t] = exp(-0.1*t*theta_m) is a
  geometric sequence, so the FFT "full" convolution with the AIF evaluated
  only at the 50 output sample indices is a small Toeplitz matmul
     U[k, n] = sum_t A[k, t] * em[t, n]        (A from the AIF, host-built)
  and the he/hp normalizations collapse into per-pixel scalars:
     conc[k, n] = c1[n]*U[k, n] + c2[n]*V[k, n]
  with c1 = ve/(Sm-Sp) + vp*alpha/(alpha*Sm+beta*Sp),
       c2 = -ve/(Sm-Sp) + vp*beta/(alpha*Sm+beta*Sp),
       alpha = 1-Te*theta_m, beta = Te*theta_p-1,
       Sm = (1-rm^L)/(1-rm), rm = exp(-0.1*theta_m)  (geometric closed form).
  Finally the SPGR signal model is applied elementwise.

Sharding: 320x320 = 102400 pixels, data-parallel: 12800 pixels per core.
Device layout per core: per-pixel prep in a [25, 512] "rows" layout bounced
through DRAM; per pixel-group, rows are broadcast-DMA'd to 128 partitions,
ACT computes em/ep in [128 t, W n] chunks (per-partition scale = -0.1*t),
PE contracts the 5 t-chunks against the AIF Toeplitz matrix (float32r
moving mode, full fp32 precision at 1 cycle/row) into PSUM, and the
epilogue is split across DVE and Pool.
"""

import os
from contextlib import ExitStack

import numpy as np

H = W = 320
NPIX = H * W
NCORES = 8
SHARD = NPIX // NCORES      # 12800 pixels per core
NT = 512                    # pixels per PSUM subtile (matmul N, one bank)
NTILES = SHARD // NT        # 25
GROUP = 2                   # pixel-subtiles per exp/broadcast group
TS = 50                     # output time samples
STEP = 0.1
DELAY = 30                  # 3s bolus delay in fine-grid samples
KP = 128                    # partition chunk of the fine time grid
KCH = 5                     # ceil(L / 128) with L = 589

# SPGR constants (from reference.py)
SIG_BASELINE = 100.0
R1 = 1.0
R1CA = 4.3
FA = 10.0
TR = 0.00487

_CACHE: dict = {}


def _fine_grid(sample_time: np.ndarray):
    t_end = float(np.asarray(sample_time)[-1])
    L = int(round(t_end / STEP)) + 1
    # f32 multiply to match jnp.arange(L)*0.1 rounding exactly
    t_samp = np.arange(L, dtype=np.float32) * np.float32(STEP)
    return L, t_samp


def _build_bass():
    import concourse.bass as bass
    import concourse.tile as tile
    from concourse import bacc, mybir

    f32 = mybir.dt.float32
    f32r = mybir.dt.float32r
    AF = mybir.ActivationFunctionType
    ALU = mybir.AluOpType

    nc = bacc.Bacc()
    pmap = nc.dram_tensor("pmap", [4, SHARD], f32, kind="ExternalInput")
    amat = nc.dram_tensor("amat", [KP, KCH, TS], f32, kind="ExternalInput")
    svec = nc.dram_tensor("svec", [KP, KCH], f32, kind="ExternalInput")
    sig = nc.dram_tensor("sig", [TS, SHARD], f32, kind="ExternalOutput")

    fa = FA * np.pi / 180.0
    cosf = float(np.cos(np.float32(fa)))
    sinf = float(np.sin(np.float32(fa)))
    E1 = float(np.exp(np.float32(-TR * R1)))
    M0 = SIG_BASELINE * (1.0 - cosf * E1) / (sinf * (1.0 - E1))
    M0t = M0 * sinf
    M_steady = M0t * (1.0 - E1) / (1.0 - E1 * cosf)
    C0 = SIG_BASELINE - M_steady
    L = 589

    groups = []
    j = 0
    while j < NTILES:
        b = min(GROUP, NTILES - j)
        groups.append((j, b))
        j += b
    WMAX = GROUP * NT

    with tile.TileContext(nc) as tc, ExitStack() as ctx:
        # Loop pools first: their SBUF must not alias the prep pool, or the
        # first broadcast would wait for the whole prep phase to release.
        const = ctx.enter_context(tc.tile_pool(name="const", bufs=1))
        bcast = ctx.enter_context(tc.tile_pool(name="bcast", bufs=3))
        empool = ctx.enter_context(tc.tile_pool(name="em", bufs=8))
        psum = ctx.enter_context(
            tc.tile_pool(name="psum", bufs=2, space=bass.MemorySpace.PSUM)
        )
        epi = ctx.enter_context(tc.tile_pool(name="epi", bufs=2))
        prep = ctx.enter_context(tc.tile_pool(name="prep", bufs=1))
        dpool = ctx.enter_context(tc.tile_pool(name="drows", bufs=1, space="DRAM"))

        a_sb = const.tile([KP, KCH, TS], f32, tag="a_sb", name="a_sb")
        a_r = const.tile([KP, KCH, TS], f32r, tag="a_r", name="a_r")
        bias_sb = const.tile([TS, 1], f32, tag="bias_sb", name="bias_sb")
        nc.vector.memset(bias_sb, float(-TR * R1))
        nc.sync.dma_start(out=a_sb[:], in_=amat[:])
        nc.vector.tensor_copy(a_r[:], a_sb[:])
        sv_sb = const.tile([KP, KCH], f32, tag="sv_sb", name="sv_sb")
        nc.sync.dma_start(out=sv_sb[:], in_=svec[:])

        # ---- per-pixel prep in rows layout [NTILES, NT] ----
        def ptile(tag):
            return prep.tile([NTILES, NT], f32, tag=tag, name=tag)

        ve, vp, fp, ps = (ptile(t) for t in ("ve", "vp", "fp", "ps"))
        for i, t in enumerate((ve, vp, fp, ps)):
            nc.sync.dma_start(
                out=t, in_=pmap[i, :].rearrange("(j q) -> j q", j=NTILES)
            )

        V = nc.vector
        G = nc.gpsimd
        # critical chain to theta_m / theta_p, DVE only
        rfp = ptile("rfp"); V.reciprocal_approx_fast(rfp, fp)
        rps = ptile("rps"); V.reciprocal_approx_fast(rps, ps)
        Te = ptile("Te"); V.tensor_mul(Te, ve, rps)
        s_ = ptile("s_"); V.tensor_add(s_, vp, ve)
        T_ = ptile("T_"); V.tensor_mul(T_, s_, rfp)           # (vp+ve)/fp
        Tc = ptile("Tc"); G.tensor_mul(Tc, vp, rfp)           # pool branch
        V.tensor_add(s_, T_, Te)                               # s = T+Te
        m4 = ptile("m4")
        V.scalar_tensor_tensor(m4, Tc, 4.0, Te, op0=ALU.mult, op1=ALU.mult)
        sq = ptile("sq"); V.tensor_mul(sq, s_, s_)
        V.tensor_sub(sq, sq, m4)                               # d^2
        disc = T_  # T_ dead after s; reuse slot
        nc.scalar.sqrt(disc, sq)
        den = Tc   # Tc dead after m4; reuse slot
        V.tensor_add(den, s_, disc)
        rden = ptile("rden"); V.reciprocal_approx_fast(rden, den)
        thm = ptile("thm"); V.tensor_scalar_mul(thm, rden, 2.0)
        rm4 = ptile("rm4"); V.reciprocal_approx_fast(rm4, m4)
        thp = ptile("thp")
        V.scalar_tensor_tensor(thp, den, 2.0, rm4, op0=ALU.mult, op1=ALU.mult)

        rows_names = ["r_thm", "r_thp", "r_c1", "r_c2", "r_w"]
        rows_d = [
            dpool.tile([NTILES, NT], f32, tag=n, name=n) for n in rows_names
        ]
        nc.sync.dma_start(out=rows_d[0][:], in_=thm)
        nc.sync.dma_start(out=rows_d[1][:], in_=thp)
        w256 = ptile("w256")
        nc.scalar.activation(w256, thm, AF.Exp, bias=0.0, scale=-STEP * 256.0)
        nc.sync.dma_start(out=rows_d[4][:], in_=w256)


        # tail of prep (c1/c2), spread DVE/pool; bounced on the SWDGE queue
        alp = ptile("alp"); G.tensor_mul(alp, Te, thm)
        G.tensor_scalar(alp, alp, -1.0, 1.0, op0=ALU.mult, op1=ALU.add)
        bet = ptile("bet"); G.tensor_mul(bet, Te, thp)
        G.tensor_scalar_sub(bet, bet, 1.0)

        def geo(theta, tag):
            r1 = ptile(tag + "_r1")
            nc.scalar.activation(r1, theta, AF.Exp, bias=0.0, scale=-STEP)
            rl = ptile(tag + "_rl")
            nc.scalar.activation(rl, theta, AF.Exp, bias=0.0, scale=-STEP * L)
            V.tensor_scalar(rl, rl, -1.0, 1.0, op0=ALU.mult, op1=ALU.add)
            V.tensor_scalar(r1, r1, -1.0, 1.0, op0=ALU.mult, op1=ALU.add)
            V.reciprocal_approx_fast(r1, r1)
            V.tensor_mul(rl, rl, r1)
            return rl                                          # S

        Sm = geo(thm, "gm")
        Sp = geo(thp, "gp")

        de = ptile("de"); V.tensor_sub(de, Sm, Sp)
        V.reciprocal_approx_fast(de, de)                       # 1/(Sm-Sp)
        V.tensor_mul(Sm, alp, Sm)                              # a*Sm
        V.tensor_mul(Sp, bet, Sp)                              # b*Sp
        V.tensor_add(Sm, Sm, Sp)                               # dp
        V.reciprocal_approx_fast(Sm, Sm)                       # 1/(a*Sm+b*Sp)
        u_ = rden  # rden dead after thm; reuse slot
        V.tensor_mul(u_, ve, de)             # ve/(Sm-Sp)
        V.tensor_mul(alp, alp, Sm)
        c1 = ptile("c1"); V.tensor_mul(c1, vp, alp)
        V.tensor_add(c1, c1, u_)
        V.tensor_mul(bet, bet, Sm)
        c2 = ptile("c2"); V.tensor_mul(c2, vp, bet)
        V.tensor_sub(c2, c2, u_)
        nc.gpsimd.dma_start(out=rows_d[2][:], in_=c1)
        nc.gpsimd.dma_start(out=rows_d[3][:], in_=c2)

        # ---- main loop over pixel groups ----
        for j0, b in groups:
            W_ = b * NT

            def row_bcast(q, nparts):
                sl = rows_d[q][j0, :]
                return bass.AP(
                    tensor=sl.tensor, offset=sl.offset, ap=[[0, nparts], [1, W_]]
                )

            thm_bc = bcast.tile([KP, WMAX], f32, tag="thm_bc", name="thm_bc")
            nc.sync.dma_start(out=thm_bc[:, :W_], in_=row_bcast(0, KP))
            thp_bc = bcast.tile([KP, WMAX], f32, tag="thp_bc", name="thp_bc")
            nc.sync.dma_start(out=thp_bc[:, :W_], in_=row_bcast(1, KP))
            w_bc = bcast.tile([KP, WMAX], f32, tag="w_bc", name="w_bc")
            nc.sync.dma_start(out=w_bc[:, :W_], in_=row_bcast(4, KP))

            c1_bc = bcast.tile([TS, WMAX], f32, tag="c1_bc", name="c1_bc")
            nc.sync.dma_start(out=c1_bc[:, :W_], in_=row_bcast(2, TS))
            c2_bc = bcast.tile([TS, WMAX], f32, tag="c2_bc", name="c2_bc")
            nc.sync.dma_start(out=c2_bc[:, :W_], in_=row_bcast(3, TS))

            Us = [psum.tile([TS, NT], f32, tag=f"U{h}", name=f"U{h}") for h in range(b)]
            Vs = [psum.tile([TS, NT], f32, tag=f"V{h}", name=f"V{h}") for h in range(b)]
            em_hist = {}
            for tcx in range(KCH):
                em = empool.tile([KP, WMAX], f32r, tag="em", name="em")
                if tcx < 3:
                    nc.scalar.activation(
                        em[:, :W_], thm_bc[:, :W_], AF.Exp,
                        bias=0.0, scale=sv_sb[:, tcx : tcx + 1],
                    )
                else:
                    V.tensor_mul(em[:, :W_], em_hist[tcx - 2][:, :W_].bitcast(f32), w_bc[:, :W_])
                em_hist[tcx] = em
                ep = empool.tile([KP, WMAX], f32r, tag="ep", name="ep")
                nc.scalar.activation(
                    ep[:, :W_], thp_bc[:, :W_], AF.Exp,
                    bias=0.0, scale=sv_sb[:, tcx : tcx + 1],
                )
                for h in range(b):
                    lo = h * NT
                    nc.tensor.matmul(
                        Us[h],
                        a_r[:, tcx, :],
                        em[:, lo : lo + NT],
                        start=(tcx == 0), stop=(tcx == KCH - 1),
                    )
                    nc.tensor.matmul(
                        Vs[h],
                        a_r[:, tcx, :],
                        ep[:, lo : lo + NT],
                        start=(tcx == 0), stop=(tcx == KCH - 1),
                    )

            t0 = epi.tile([TS, WMAX], f32, tag="t0", name="t0")
            conc = epi.tile([TS, WMAX], f32, tag="conc", name="conc")
            for h in range(b):
                lo = h * NT
                V.tensor_mul(t0[:, lo : lo + NT], Us[h], c1_bc[:, lo : lo + NT])
                t3 = epi.tile([TS, NT], f32, tag="t3", name="t3")
                V.tensor_mul(t3, Vs[h], c2_bc[:, lo : lo + NT])
                nc.gpsimd.tensor_add(
                    conc[:, lo : lo + NT], t0[:, lo : lo + NT], t3
                )

            E = epi.tile([TS, WMAX], f32, tag="E", name="E")
            nc.scalar.activation(
                E[:, :W_], conc[:, :W_], AF.Exp,
                bias=bias_sb, scale=float(-TR * R1CA),
            )
            dn = conc  # conc is dead once E is computed; reuse its buffer
            nc.gpsimd.tensor_scalar(
                dn[:, :W_], E[:, :W_], -cosf, 1.0, op0=ALU.mult, op1=ALU.add
            )
            V.reciprocal_approx_fast(dn[:, :W_], dn[:, :W_])
            V.scalar_tensor_tensor(
                E[:, :W_], E[:, :W_], 1.0, dn[:, :W_],
                op0=ALU.subtract, op1=ALU.mult,
            )
            nc.gpsimd.tensor_scalar(
                t0[:, :W_], E[:, :W_], -M0t, C0, op0=ALU.mult, op1=ALU.add
            )
            nc.sync.dma_start(
                out=sig[:, j0 * NT : j0 * NT + W_], in_=t0[:, :W_]
            )

    nc.compile()
    return nc


def _host_prep(sample_time: np.ndarray, Cp: np.ndarray):
    """Build the AIF Toeplitz matrix A (lhsT layout) and the time-scale vectors."""
    L, t_samp = _fine_grid(sample_time)
    aifci = np.interp(
        t_samp.astype(np.float64),
        np.asarray(sample_time, np.float64),
        np.asarray(Cp, np.float64),
    ).astype(np.float32)
    aif = np.concatenate([np.zeros(DELAY, np.float32), aifci[:-DELAY]])
    idx = np.searchsorted(t_samp, np.asarray(sample_time, np.float32), side="left")
    idx = np.minimum(idx, L - 1)  # jax clamps out-of-bounds gather indices

    # A[k, t] = aif[idx[k] - t] for t <= idx[k], zero otherwise; t padded to 640
    A = np.zeros((TS, KP * KCH), np.float32)
    for k in range(TS):
        i = int(idx[k])
        A[k, : i + 1] = aif[i::-1]
    # lhsT layout: amat[p, tc, m] = A[m, tc*128 + p]
    amat = A.reshape(TS, KCH, KP).transpose(2, 1, 0).copy()

    svec = np.empty((KP, KCH), np.float32)
    for tcx in range(KCH):
        svec[:, tcx] = -STEP * (tcx * KP + np.arange(KP))
    return amat, svec


def kernel(param: np.ndarray, sample_time: np.ndarray, Cp: np.ndarray) -> np.ndarray:
    from concourse.bass_utils import run_bass_kernel_spmd

    if "nc" not in _CACHE:
        _CACHE["nc"] = _build_bass()
    nc = _CACHE["nc"]

    amat, svec = _host_prep(sample_time, Cp)
    pflat = np.ascontiguousarray(np.asarray(param, np.float32).reshape(4, NPIX))
    in_maps = []
    for c in range(NCORES):
        in_maps.append(
            {
                "pmap": np.ascontiguousarray(pflat[:, c * SHARD : (c + 1) * SHARD]),
                "amat": amat,
                "svec": svec,
            }
        )
    res = run_bass_kernel_spmd(
        nc,
        in_maps,
        core_ids=list(range(NCORES)),
        trace=bool(int(os.environ.get("DCE_TRACE", "0"))),
    )
    if res.exec_time_ns is not None:
        _CACHE["exec_time_ns"] = res.exec_time_ns
    out = np.concatenate([r["sig"] for r in res.results], axis=1)
    return out.reshape(TS, 1, H, W)



# revision 7
# speedup vs baseline: 3.8994x; 3.8994x over previous
"""Trainium2 Bass kernel for the DCE (dynamic contrast-enhanced) 2CXM signal model.

Algorithmic core (replaces the 640-step FFT convolution of the reference):

  The sampled convolution response is, per pixel, p_k(theta) =
  sum_t A[k,t] e^{-0.1 t theta} evaluated at theta_m / theta_p -- a Laplace-
  type function of a single scalar.  It is approximated to ~1e-6 relative
  (vs a 2e-2 tolerance) by a J=32 sum of exponentials
      p_k(theta) ~= sum_j B[k,j] e^{-alpha_j theta}
  with alpha_0 = 0 and alpha_1..31 geometrically spaced; B is fitted on the
  host by ridge least squares over theta in [0.02, 64] (the attainable range
  for param in [0.05, 1]^4 is well inside).  conc = c1*p(theta_m) +
  c2*p(theta_p) with the same per-pixel c1/c2 closed forms as before.

  The SPGR epilogue uses the exact identity
      1/(1 - c e^{-u}) = (coth(u/2) + 1)/2,  u = TR*(R1 + R1CA*conc) - ln c
  so sig = (K2/2)/tanh(u/2) + (K1 + K2/2): one Tanh, one reciprocal, one
  affine.  Exp and Tanh share one ACT table set (no in-loop table loads).

Device layout (per core, 12800 pixels):
  - prep in pixel-partition layout [128, 100] (pixel = p*100 + q):
    theta_m/theta_p/c1/c2 via ~35 elementwise ops, then 8 small reshape DMAs
    into a rows8 [8, 6400] SBUF tensor (rows: thm/thp/c1/c2 x half0/half1).
  - main loop over 13 pixel groups (12x512 + 256), 4-way stacked tiles
    [128 = 4 blocks x 32 alphas, W2]: blocks (em half0, ep half0, em half1,
    ep half1).  PE ones-matmul broadcasts rows -> PSUM, ACT computes the
    exp basis in ONE instruction per group (per-partition scale = -alpha),
    DVE applies the c1/c2 scaling, and ONE matmul against the block lhsT
    B4 [128, 100] contracts basis -> conc for both halves at once
    ([100, W2] PSUM: rows 0-49 half0, 50-99 half1).
  - epilogue: ACT Tanh, DVE reciprocal, Pool affine, DMA out every 4 groups.
"""

import os

import numpy as np

H = W = 320
NPIX = H * W
NCORES = 8
SHARD = NPIX // NCORES      # 12800 pixels per core
HALF = SHARD // 2           # 6400 (stacking half)
QP = 100                    # free size of the [128, 100] prep layout
W2 = 512                    # pixels per half per group (PSUM bank = 512 f32)
NG = (HALF + W2 - 1) // W2  # 13 groups (12 x 512 + 1 x 256)
J = 32                      # exponential-basis size
L = 589                     # fine time-grid length
TS = 50                     # output time samples
STEP = 0.1
DELAY = 30                  # 3s bolus delay in fine-grid samples
FLUSH = 4                   # groups per output DMA

# SPGR constants (from reference.py)
SIG_BASELINE = 100.0
R1 = 1.0
R1CA = 4.3
FA = 10.0
TR = 0.00487

_CACHE: dict = {}


def _spgr_consts():
    fa = FA * np.pi / 180.0
    cosf = float(np.cos(np.float32(fa)))
    sinf = float(np.sin(np.float32(fa)))
    E1 = float(np.exp(np.float32(-TR * R1)))
    M0 = SIG_BASELINE * (1.0 - cosf * E1) / (sinf * (1.0 - E1))
    M0t = M0 * sinf
    M_steady = M0t * (1.0 - E1) / (1.0 - E1 * cosf)
    C0 = SIG_BASELINE - M_steady
    K1 = M0t / cosf + C0
    K2 = -M0t * (1.0 - cosf) / cosf
    a = TR * R1CA
    b = TR * R1 - float(np.log(cosf))
    return K1, K2, a, b


def _alphas():
    return np.concatenate(
        [[0.0], np.geomspace(0.05, 58.8, J - 1)]
    ).astype(np.float64)


def _build_bass():
    import concourse.bass as bass
    import concourse.tile as tile
    from concourse import bacc, mybir
    from contextlib import ExitStack

    f32 = mybir.dt.float32
    f32r = mybir.dt.float32r
    AF = mybir.ActivationFunctionType
    ALU = mybir.AluOpType

    K1, K2, a_, b_ = _spgr_consts()

    nc = bacc.Bacc()
    pmap = nc.dram_tensor("pmap", [4, SHARD], f32, kind="ExternalInput")
    b4d = nc.dram_tensor("b4", [128, 2 * TS], f32, kind="ExternalInput")
    ones4d = nc.dram_tensor("ones4", [4, 128], f32, kind="ExternalInput")
    svd = nc.dram_tensor("sv", [128, 1], f32, kind="ExternalInput")
    sig2 = nc.dram_tensor("sig2", [2 * TS, HALF], f32, kind="ExternalOutput")

    with tile.TileContext(nc) as tc, ExitStack() as ctx:
        const = ctx.enter_context(tc.tile_pool(name="const", bufs=1))
        rows = ctx.enter_context(tc.tile_pool(name="rows", bufs=1))
        ebp = ctx.enter_context(tc.tile_pool(name="ebp", bufs=3))
        rhp = ctx.enter_context(tc.tile_pool(name="rhp", bufs=3))
        epi = ctx.enter_context(tc.tile_pool(name="epi", bufs=3))
        obp = ctx.enter_context(tc.tile_pool(name="obp", bufs=2))
        psbc = ctx.enter_context(
            tc.tile_pool(name="psbc", bufs=2, space=bass.MemorySpace.PSUM)
        )
        pcc = ctx.enter_context(
            tc.tile_pool(name="pcc", bufs=3, space=bass.MemorySpace.PSUM)
        )
        prep = ctx.enter_context(tc.tile_pool(name="prep", bufs=1))

        # ---- constants ----
        b4_sb = const.tile([128, 2 * TS], f32, tag="b4_sb", name="b4_sb")
        b4_r = const.tile([128, 2 * TS], f32r, tag="b4_r", name="b4_r")
        o4_sb = const.tile([4, 128], f32, tag="o4_sb", name="o4_sb")
        o4_r = const.tile([4, 128], f32r, tag="o4_r", name="o4_r")
        sv_sb = const.tile([128, 1], f32, tag="sv_sb", name="sv_sb")
        tb_sb = const.tile([2 * TS, 1], f32, tag="tb_sb", name="tb_sb")
        nc.sync.dma_start(out=b4_sb[:], in_=b4d[:])
        nc.sync.dma_start(out=o4_sb[:], in_=ones4d[:])
        nc.sync.dma_start(out=sv_sb[:], in_=svd[:])
        nc.vector.tensor_copy(b4_r[:], b4_sb[:])
        nc.vector.tensor_copy(o4_r[:], o4_sb[:])

        V = nc.vector
        G = nc.gpsimd

        # ---- per-pixel prep, [128, 100] pixel-partition layout ----
        def ptile(tag):
            return prep.tile([128, QP], f32, tag=tag, name=tag)

        ve, vp, fp, ps = (ptile(t) for t in ("ve", "vp", "fp", "ps"))
        for i, t in enumerate((ve, vp, fp, ps)):
            nc.sync.dma_start(
                out=t, in_=pmap[i, :].rearrange("(p q) -> p q", p=128)
            )

        thmthp = prep.tile([128, 2 * QP], f32, tag="thmthp", name="thmthp")
        c1c2 = prep.tile([128, 2 * QP], f32, tag="c1c2", name="c1c2")

        rfp = ptile("rfp"); V.reciprocal_approx_fast(rfp, fp)
        rps = ptile("rps"); V.reciprocal_approx_fast(rps, ps)
        Te = ptile("Te"); V.tensor_mul(Te, ve, rps)
        sv2 = ptile("sv2"); G.tensor_add(sv2, vp, ve)
        T_ = ptile("T_"); V.tensor_mul(T_, sv2, rfp)
        Tc = ptile("Tc"); G.tensor_mul(Tc, vp, rfp)
        s_ = ptile("s_"); V.tensor_add(s_, T_, Te)
        q4 = ptile("q4")
        V.scalar_tensor_tensor(q4, Tc, 4.0, Te, op0=ALU.mult, op1=ALU.mult)
        sq = ptile("sq"); V.tensor_mul(sq, s_, s_)
        V.tensor_sub(sq, sq, q4)
        d_ = ptile("d_"); nc.scalar.sqrt(d_, sq)
        den = ptile("den"); V.tensor_add(den, s_, d_)
        rden = ptile("rden"); V.reciprocal_approx_fast(rden, den)
        thm = thmthp[:, 0:QP]
        V.tensor_scalar_mul(thm, rden, 2.0)
        rq4 = ptile("rq4"); V.reciprocal_approx_fast(rq4, q4)
        thp = thmthp[:, QP : 2 * QP]
        V.scalar_tensor_tensor(thp, den, 2.0, rq4, op0=ALU.mult, op1=ALU.mult)

        # geometric-sum normalizers Sm/Sp (sum over t=0..588 of r^t)
        e1m = ptile("e1m"); nc.scalar.activation(e1m, thm, AF.Exp, bias=0.0, scale=-STEP)
        eLm = ptile("eLm"); nc.scalar.activation(eLm, thm, AF.Exp, bias=0.0, scale=-STEP * L)
        e1p = ptile("e1p"); nc.scalar.activation(e1p, thp, AF.Exp, bias=0.0, scale=-STEP)
        eLp = ptile("eLp"); nc.scalar.activation(eLp, thp, AF.Exp, bias=0.0, scale=-STEP * L)
        nm = ptile("nm"); G.tensor_scalar(nm, eLm, -1.0, 1.0, op0=ALU.mult, op1=ALU.add)
        dm = ptile("dm"); V.tensor_scalar(dm, e1m, -1.0, 1.0, op0=ALU.mult, op1=ALU.add)
        V.reciprocal_approx_fast(dm, dm)
        Sm = ptile("Sm"); V.tensor_mul(Sm, nm, dm)
        np_ = ptile("np_"); G.tensor_scalar(np_, eLp, -1.0, 1.0, op0=ALU.mult, op1=ALU.add)
        dp2 = ptile("dp2"); V.tensor_scalar(dp2, e1p, -1.0, 1.0, op0=ALU.mult, op1=ALU.add)
        V.reciprocal_approx_fast(dp2, dp2)
        Sp = ptile("Sp"); V.tensor_mul(Sp, np_, dp2)

        um = ptile("um"); G.tensor_mul(um, Te, thm)
        alp = ptile("alp"); G.tensor_scalar(alp, um, -1.0, 1.0, op0=ALU.mult, op1=ALU.add)
        up = ptile("up"); G.tensor_mul(up, Te, thp)
        bet = ptile("bet"); G.tensor_scalar_sub(bet, up, 1.0)

        dS = ptile("dS"); V.tensor_sub(dS, Sm, Sp)
        V.reciprocal_approx_fast(dS, dS)
        vede = ptile("vede"); V.tensor_mul(vede, ve, dS)
        aS = ptile("aS"); G.tensor_mul(aS, alp, Sm)
        bS = ptile("bS"); G.tensor_mul(bS, bet, Sp)
        ab = ptile("ab"); V.tensor_add(ab, aS, bS)
        V.reciprocal_approx_fast(ab, ab)
        w1 = ptile("w1"); G.tensor_mul(w1, vp, alp)
        V.tensor_mul(w1, w1, ab)
        c1s = c1c2[:, 0:QP]
        V.tensor_add(c1s, w1, vede)
        w2 = ptile("w2"); G.tensor_mul(w2, vp, bet)
        V.tensor_mul(w2, w2, ab)
        c2s = c1c2[:, QP : 2 * QP]
        V.tensor_sub(c2s, w2, vede)

        # ---- reshape to rows8 [8, 6400]: (thm,thp,c1,c2) x (half0,half1) ----
        rowsT = rows.tile([4, HALF], f32, tag="rowsT", name="rowsT")
        rowsC = rows.tile([4, HALF], f32, tag="rowsC", name="rowsC")
        for hh in range(2):
            psl = slice(hh * 64, (hh + 1) * 64)
            nc.sync.dma_start(
                out=rowsT[2 * hh : 2 * hh + 1, :], in_=thmthp[psl, 0:QP]
            )
            nc.sync.dma_start(
                out=rowsT[2 * hh + 1 : 2 * hh + 2, :], in_=thmthp[psl, QP : 2 * QP]
            )
            nc.scalar.dma_start(
                out=rowsC[2 * hh : 2 * hh + 1, :], in_=c1c2[psl, 0:QP]
            )
            nc.scalar.dma_start(
                out=rowsC[2 * hh + 1 : 2 * hh + 2, :], in_=c1c2[psl, QP : 2 * QP]
            )

        # ---- main loop over pixel groups ----
        tanh_scale = float(a_ / 2.0)
        nc.vector.memset(tb_sb, float(b_ / 2.0))
        fs1 = float(K2 / 2.0)
        fs2 = float(K1 + K2 / 2.0)

        obuf = None
        ob0 = 0
        for g in range(NG):
            lo = g * W2
            W_ = min(W2, HALF - lo)
            sl = slice(lo, lo + W_)

            th_bc = psbc.tile([128, W2], f32, tag="th_bc", name="th_bc")
            nc.tensor.matmul(
                th_bc[:, :W_], o4_r[:], rowsT[:, sl].bitcast(f32r),
                start=True, stop=True,
            )
            eb = ebp.tile([128, W2], f32, tag="eb", name="eb")
            nc.scalar.activation(
                eb[:, :W_], th_bc[:, :W_], AF.Exp, bias=0.0, scale=sv_sb[:, 0:1]
            )
            c_bc = psbc.tile([128, W2], f32, tag="c_bc", name="c_bc")
            nc.tensor.matmul(
                c_bc[:, :W_], o4_r[:], rowsC[:, sl].bitcast(f32r),
                start=True, stop=True,
            )
            rhs = rhp.tile([128, W2], f32r, tag="rhs", name="rhs")
            V.tensor_mul(rhs[:, :W_], eb[:, :W_], c_bc[:, :W_])

            conc = pcc.tile([2 * TS, W2], f32, tag="conc", name="conc")
            nc.tensor.matmul(
                conc[:, :W_], b4_r[:], rhs[:, :W_], start=True, stop=True
            )

            tht = epi.tile([2 * TS, W2], f32, tag="tht", name="tht")
            nc.scalar.activation(
                tht[:, :W_], conc[:, :W_], AF.Tanh,
                bias=tb_sb, scale=tanh_scale,
            )
            rt = epi.tile([2 * TS, W2], f32, tag="rt", name="rt")
            V.reciprocal_approx_fast(rt[:, :W_], tht[:, :W_])

            if obuf is None:
                obuf = obp.tile([2 * TS, FLUSH * W2], f32, tag="obuf", name="obuf")
                ob0 = lo
            off = lo - ob0
            G.tensor_scalar(
                obuf[:, off : off + W_], rt[:, :W_], fs1, fs2,
                op0=ALU.mult, op1=ALU.add,
            )
            if g == NG - 1 or off + W_ == FLUSH * W2:
                nc.sync.dma_start(
                    out=sig2[:, ob0 : lo + W_], in_=obuf[:, : lo + W_ - ob0]
                )
                obuf = None

    nc.compile()
    return nc


def _host_prep(sample_time: np.ndarray, Cp: np.ndarray):
    """Build the AIF response matrix A, fit the J-term exponential basis, and
    pack the block lhsT / broadcast-ones / scale constants."""
    t_end = float(np.asarray(sample_time)[-1])
    Lf = int(round(t_end / STEP)) + 1
    t_samp = np.arange(Lf, dtype=np.float32) * np.float32(STEP)
    aifci = np.interp(
        t_samp.astype(np.float64),
        np.asarray(sample_time, np.float64),
        np.asarray(Cp, np.float64),
    ).astype(np.float32)
    aif = np.concatenate([np.zeros(DELAY, np.float32), aifci[:-DELAY]])
    idx = np.searchsorted(t_samp, np.asarray(sample_time, np.float32), side="left")
    idx = np.minimum(idx, Lf - 1)

    # A[k, t] = aif[idx[k] - t] for t <= idx[k] (conv 'full' sampled at idx)
    A = np.zeros((TS, 640), np.float64)
    for k in range(TS):
        i = int(idx[k])
        A[k, : i + 1] = aif[i::-1]

    alphas = _alphas()
    tg = np.arange(640, dtype=np.float64)
    g = np.geomspace(0.02, 64.0, 4000)
    P = np.exp(-0.1 * np.outer(g, tg)) @ A.T          # [G, 50]
    M = np.exp(-np.outer(g, alphas))                  # [G, J]
    B = np.linalg.solve(M.T @ M + 1e-8 * np.eye(J), M.T @ P)  # [J, 50]
    B = B.astype(np.float32)

    # block lhsT [128, 100]: blocks (em h0, ep h0, em h1, ep h1) x 32 alphas
    b4 = np.zeros((128, 2 * TS), np.float32)
    b4[0:J, 0:TS] = B
    b4[J : 2 * J, 0:TS] = B
    b4[2 * J : 3 * J, TS : 2 * TS] = B
    b4[3 * J : 4 * J, TS : 2 * TS] = B

    ones4 = np.zeros((4, 128), np.float32)
    for r in range(4):
        ones4[r, r * J : (r + 1) * J] = 1.0

    sv = (-alphas[np.arange(128) % J]).reshape(128, 1).astype(np.float32)
    return b4, ones4, sv


def kernel(param: np.ndarray, sample_time: np.ndarray, Cp: np.ndarray) -> np.ndarray:
    from concourse.bass_utils import run_bass_kernel_spmd

    if "nc" not in _CACHE:
        _CACHE["nc"] = _build_bass()
    nc = _CACHE["nc"]

    b4, ones4, sv = _host_prep(sample_time, Cp)
    pflat = np.ascontiguousarray(np.asarray(param, np.float32).reshape(4, NPIX))
    in_maps = []
    for c in range(NCORES):
        in_maps.append(
            {
                "pmap": np.ascontiguousarray(pflat[:, c * SHARD : (c + 1) * SHARD]),
                "b4": b4,
                "ones4": ones4,
                "sv": sv,
            }
        )
    res = run_bass_kernel_spmd(
        nc,
        in_maps,
        core_ids=list(range(NCORES)),
        trace=bool(int(os.environ.get("DCE_TRACE", "0"))),
    )
    if res.exec_time_ns is not None:
        _CACHE["exec_time_ns"] = res.exec_time_ns
    # sig2 [100, 6400] per core: rows 0-49 half0 pixels, rows 50-99 half1
    parts = []
    for r in res.results:
        s2 = r["sig2"]
        parts.append(np.concatenate([s2[:TS, :], s2[TS:, :]], axis=1))
    out = np.concatenate(parts, axis=1)
    return out.reshape(TS, 1, H, W)


# revision 9
# speedup vs baseline: 4.3061x; 1.1043x over previous
"""Trainium2 Bass kernel for the DCE (dynamic contrast-enhanced) 2CXM signal model.

Algorithmic core (replaces the 640-step FFT convolution of the reference):

  The sampled convolution response is, per pixel, p_k(theta) =
  sum_t A[k,t] e^{-0.1 t theta} evaluated at theta_m / theta_p -- a Laplace-
  type function of a single scalar.  It is approximated to ~1e-6 relative
  (vs a 2e-2 tolerance) by a J=32 sum of exponentials
      p_k(theta) ~= sum_j B[k,j] e^{-alpha_j theta}
  with alpha_0 = 0 and alpha_1..31 geometrically spaced; B is fitted on the
  host by ridge least squares over theta in [0.02, 64] (the attainable range
  for param in [0.05, 1]^4 is well inside).  conc = c1*p(theta_m) +
  c2*p(theta_p) with the same per-pixel c1/c2 closed forms as before.

  The SPGR epilogue uses the exact identity
      1/(1 - c e^{-u}) = (coth(u/2) + 1)/2,  u = TR*(R1 + R1CA*conc) - ln c
  so sig = (K2/2)/tanh(u/2) + (K1 + K2/2): one Tanh, one reciprocal, one
  affine.  Exp and Tanh share one ACT table set (no in-loop table loads).

Device layout (per core, 12800 pixels):
  - prep in pixel-partition layout [128, 100] (pixel = p*100 + q):
    theta_m/theta_p/c1/c2 via ~35 elementwise ops, then 8 small reshape DMAs
    into a rows8 [8, 6400] SBUF tensor (rows: thm/thp/c1/c2 x half0/half1).
  - main loop over 13 pixel groups (12x512 + 256), 4-way stacked tiles
    [128 = 4 blocks x 32 alphas, W2]: blocks (em half0, ep half0, em half1,
    ep half1).  PE ones-matmul broadcasts rows -> PSUM, ACT computes the
    exp basis in ONE instruction per group (per-partition scale = -alpha),
    DVE applies the c1/c2 scaling, and ONE matmul against the block lhsT
    B4 [128, 100] contracts basis -> conc for both halves at once
    ([100, W2] PSUM: rows 0-49 half0, 50-99 half1).
  - epilogue: ACT Tanh, DVE reciprocal, Pool affine, DMA out every 4 groups.
"""

import os

import numpy as np

H = W = 320
NPIX = H * W
NCORES = 8
SHARD = NPIX // NCORES      # 12800 pixels per core
HALF = SHARD // 2           # 6400 (stacking half)
QP = 100                    # free size of the [128, 100] prep layout
W2 = 512                    # pixels per half per group (PSUM bank = 512 f32)
NG = (HALF + W2 - 1) // W2  # 13 groups (12 x 512 + 1 x 256)
J = 32                      # exponential-basis size
L = 589                     # fine time-grid length
TS = 50                     # output time samples
STEP = 0.1
DELAY = 30                  # 3s bolus delay in fine-grid samples
FLUSH = 1                   # groups per output DMA

# SPGR constants (from reference.py)
SIG_BASELINE = 100.0
R1 = 1.0
R1CA = 4.3
FA = 10.0
TR = 0.00487

_CACHE: dict = {}


def _spgr_consts():
    fa = FA * np.pi / 180.0
    cosf = float(np.cos(np.float32(fa)))
    sinf = float(np.sin(np.float32(fa)))
    E1 = float(np.exp(np.float32(-TR * R1)))
    M0 = SIG_BASELINE * (1.0 - cosf * E1) / (sinf * (1.0 - E1))
    M0t = M0 * sinf
    M_steady = M0t * (1.0 - E1) / (1.0 - E1 * cosf)
    C0 = SIG_BASELINE - M_steady
    K1 = M0t / cosf + C0
    K2 = -M0t * (1.0 - cosf) / cosf
    a = TR * R1CA
    b = TR * R1 - float(np.log(cosf))
    return K1, K2, a, b


def _alphas():
    return np.concatenate(
        [[0.0], np.geomspace(0.05, 58.8, J - 1)]
    ).astype(np.float64)


def _build_bass():
    import concourse.bass as bass
    import concourse.tile as tile
    from concourse import bacc, mybir
    from contextlib import ExitStack

    f32 = mybir.dt.float32
    f32r = mybir.dt.float32r
    AF = mybir.ActivationFunctionType
    ALU = mybir.AluOpType

    K1, K2, a_, b_ = _spgr_consts()

    nc = bacc.Bacc()
    pmap = nc.dram_tensor("pmap", [4, SHARD], f32, kind="ExternalInput")
    b4d = nc.dram_tensor("b4", [128, 2 * TS], f32, kind="ExternalInput")
    ones4d = nc.dram_tensor("ones4", [4, 128], f32, kind="ExternalInput")
    svd = nc.dram_tensor("sv", [128, 1], f32, kind="ExternalInput")
    sig2 = nc.dram_tensor("sig2", [2 * TS, HALF], f32, kind="ExternalOutput")

    with tile.TileContext(nc) as tc, ExitStack() as ctx:
        const = ctx.enter_context(tc.tile_pool(name="const", bufs=1))
        rows = ctx.enter_context(tc.tile_pool(name="rows", bufs=1))
        ebp = ctx.enter_context(tc.tile_pool(name="ebp", bufs=3))
        rhp = ctx.enter_context(tc.tile_pool(name="rhp", bufs=3))
        epi = ctx.enter_context(tc.tile_pool(name="epi", bufs=3))
        obp = ctx.enter_context(tc.tile_pool(name="obp", bufs=3))
        psbc = ctx.enter_context(
            tc.tile_pool(name="psbc", bufs=2, space=bass.MemorySpace.PSUM)
        )
        pcc = ctx.enter_context(
            tc.tile_pool(name="pcc", bufs=4, space=bass.MemorySpace.PSUM)
        )
        prep = ctx.enter_context(tc.tile_pool(name="prep", bufs=1))

        # ---- constants ----
        b4_sb = const.tile([128, 2 * TS], f32, tag="b4_sb", name="b4_sb")
        b4_r = const.tile([128, 2 * TS], f32r, tag="b4_r", name="b4_r")
        o4_sb = const.tile([4, 128], f32, tag="o4_sb", name="o4_sb")
        o4_r = const.tile([4, 128], f32r, tag="o4_r", name="o4_r")
        sv_sb = const.tile([128, 1], f32, tag="sv_sb", name="sv_sb")
        tb_sb = const.tile([2 * TS, 1], f32, tag="tb_sb", name="tb_sb")
        nc.scalar.dma_start(out=b4_sb[:], in_=b4d[:])
        nc.scalar.dma_start(out=o4_sb[:], in_=ones4d[:])
        nc.scalar.dma_start(out=sv_sb[:], in_=svd[:])
        nc.vector.tensor_copy(b4_r[:], b4_sb[:])
        nc.vector.tensor_copy(o4_r[:], o4_sb[:])

        V = nc.vector
        G = nc.gpsimd

        # ---- per-pixel prep, [128, 100] pixel-partition layout ----
        def ptile(tag):
            return prep.tile([128, QP], f32, tag=tag, name=tag)

        pin1 = prep.tile([128, 2, QP], f32, tag="pin1", name="pin1")
        pin2 = prep.tile([128, 2, QP], f32, tag="pin2", name="pin2")
        # pin1 = (fp, ps), pin2 = (ve, vp): chain can start after pin1 lands
        nc.sync.dma_start(
            out=pin1, in_=pmap[2:4, :].rearrange("v (p q) -> p v q", p=128)
        )
        nc.sync.dma_start(
            out=pin2, in_=pmap[0:2, :].rearrange("v (p q) -> p v q", p=128)
        )
        fp = pin1[:, 0, :]; ps = pin1[:, 1, :]
        ve = pin2[:, 0, :]; vp = pin2[:, 1, :]

        thmthp = prep.tile([128, 2 * QP], f32, tag="thmthp", name="thmthp")
        c1c2 = prep.tile([128, 2 * QP], f32, tag="c1c2", name="c1c2")

        rfp = ptile("rfp"); V.reciprocal_approx_fast(rfp, fp)
        rps = ptile("rps"); V.reciprocal_approx_fast(rps, ps)
        Te = ptile("Te"); V.tensor_mul(Te, ve, rps)
        sv2 = ptile("sv2"); V.tensor_add(sv2, vp, ve)
        T_ = ptile("T_"); V.tensor_mul(T_, sv2, rfp)
        Tc = ptile("Tc"); G.tensor_mul(Tc, vp, rfp)
        s_ = ptile("s_"); V.tensor_add(s_, T_, Te)
        q4 = ptile("q4")
        V.scalar_tensor_tensor(q4, Tc, 4.0, Te, op0=ALU.mult, op1=ALU.mult)
        sq = ptile("sq"); V.tensor_mul(sq, s_, s_)
        V.tensor_sub(sq, sq, q4)
        d_ = ptile("d_"); nc.scalar.sqrt(d_, sq)
        den = ptile("den"); V.tensor_add(den, s_, d_)
        rden = ptile("rden"); V.reciprocal_approx_fast(rden, den)
        thm = thmthp[:, 0:QP]
        V.tensor_scalar_mul(thm, rden, 2.0)
        rq4 = ptile("rq4"); V.reciprocal_approx_fast(rq4, q4)
        thp = thmthp[:, QP : 2 * QP]
        V.scalar_tensor_tensor(thp, den, 2.0, rq4, op0=ALU.mult, op1=ALU.mult)

        # theta rows can ship as soon as thm/thp are written
        rowsT = rows.tile([4, HALF], f32, tag="rowsT", name="rowsT")
        rowsC = rows.tile([4, HALF], f32, tag="rowsC", name="rowsC")
        nc.sync.dma_start(out=rowsT[0:2, :], in_=thmthp[:, 0:QP])
        nc.sync.dma_start(out=rowsT[2:4, :], in_=thmthp[:, QP : 2 * QP])

        # geometric-sum normalizers Sm/Sp (sum over t=0..588 of r^t)
        e1m = ptile("e1m"); nc.scalar.activation(e1m, thm, AF.Exp, bias=0.0, scale=-STEP)
        eLm = ptile("eLm"); nc.scalar.activation(eLm, thm, AF.Exp, bias=0.0, scale=-STEP * L)
        e1p = ptile("e1p"); nc.scalar.activation(e1p, thp, AF.Exp, bias=0.0, scale=-STEP)
        eLp = ptile("eLp"); nc.scalar.activation(eLp, thp, AF.Exp, bias=0.0, scale=-STEP * L)
        nm = ptile("nm"); G.tensor_scalar(nm, eLm, -1.0, 1.0, op0=ALU.mult, op1=ALU.add)
        dm = ptile("dm"); V.tensor_scalar(dm, e1m, -1.0, 1.0, op0=ALU.mult, op1=ALU.add)
        V.reciprocal_approx_fast(dm, dm)
        Sm = ptile("Sm"); V.tensor_mul(Sm, nm, dm)
        np_ = ptile("np_"); G.tensor_scalar(np_, eLp, -1.0, 1.0, op0=ALU.mult, op1=ALU.add)
        dp2 = ptile("dp2"); V.tensor_scalar(dp2, e1p, -1.0, 1.0, op0=ALU.mult, op1=ALU.add)
        V.reciprocal_approx_fast(dp2, dp2)
        Sp = ptile("Sp"); V.tensor_mul(Sp, np_, dp2)

        um = ptile("um"); G.tensor_mul(um, Te, thm)
        alp = ptile("alp"); G.tensor_scalar(alp, um, -1.0, 1.0, op0=ALU.mult, op1=ALU.add)
        up = ptile("up"); G.tensor_mul(up, Te, thp)
        bet = ptile("bet"); G.tensor_scalar_sub(bet, up, 1.0)

        dS = ptile("dS"); V.tensor_sub(dS, Sm, Sp)
        V.reciprocal_approx_fast(dS, dS)
        vede = ptile("vede"); V.tensor_mul(vede, ve, dS)
        aS = ptile("aS"); G.tensor_mul(aS, alp, Sm)
        bS = ptile("bS"); G.tensor_mul(bS, bet, Sp)
        ab = ptile("ab"); V.tensor_add(ab, aS, bS)
        V.reciprocal_approx_fast(ab, ab)
        w1 = ptile("w1"); G.tensor_mul(w1, vp, alp)
        V.tensor_mul(w1, w1, ab)
        c1s = c1c2[:, 0:QP]
        V.tensor_add(c1s, w1, vede)
        w2 = ptile("w2"); G.tensor_mul(w2, vp, bet)
        V.tensor_mul(w2, w2, ab)
        c2s = c1c2[:, QP : 2 * QP]
        V.tensor_sub(c2s, w2, vede)

        # ---- c rows ----
        nc.scalar.dma_start(out=rowsC[0:2, :], in_=c1c2[:, 0:QP])
        nc.scalar.dma_start(out=rowsC[2:4, :], in_=c1c2[:, QP : 2 * QP])

        # ---- main loop over pixel groups ----
        tanh_scale = float(a_ / 2.0)
        nc.vector.memset(tb_sb, float(b_ / 2.0))
        fs1 = float(K2 / 2.0)
        fs2 = float(K1 + K2 / 2.0)

        for g in range(NG):
            lo = g * W2
            W_ = min(W2, HALF - lo)
            sl = slice(lo, lo + W_)

            th_bc = psbc.tile([128, W2], f32, tag="th_bc", name="th_bc")
            nc.tensor.matmul(
                th_bc[:, :W_], o4_r[:], rowsT[:, sl].bitcast(f32r),
                start=True, stop=True,
            )
            eb = ebp.tile([128, W2], f32, tag="eb", name="eb")
            nc.scalar.activation(
                eb[:, :W_], th_bc[:, :W_], AF.Exp, bias=0.0, scale=sv_sb[:, 0:1]
            )
            c_bc = psbc.tile([128, W2], f32, tag="c_bc", name="c_bc")
            nc.tensor.matmul(
                c_bc[:, :W_], o4_r[:], rowsC[:, sl].bitcast(f32r),
                start=True, stop=True,
            )
            rhs = rhp.tile([128, W2], f32r, tag="rhs", name="rhs")
            V.tensor_mul(rhs[:, :W_], eb[:, :W_], c_bc[:, :W_])

            conc = pcc.tile([2 * TS, W2], f32, tag="conc", name="conc")
            nc.tensor.matmul(
                conc[:, :W_], b4_r[:], rhs[:, :W_], start=True, stop=True
            )

            tht = epi.tile([2 * TS, W2], f32, tag="tht", name="tht")
            nc.scalar.activation(
                tht[:, :W_], conc[:, :W_], AF.Tanh,
                bias=tb_sb, scale=tanh_scale,
            )
            rt = epi.tile([2 * TS, W2], f32, tag="rt", name="rt")
            V.reciprocal_approx_fast(rt[:, :W_], tht[:, :W_])

            obuf = obp.tile([2 * TS, W2], f32, tag="obuf", name="obuf")
            G.tensor_scalar(
                obuf[:, :W_], rt[:, :W_], fs1, fs2,
                op0=ALU.mult, op1=ALU.add,
            )
            nc.sync.dma_start(out=sig2[:, sl], in_=obuf[:, :W_])

    nc.compile()
    return nc


def _host_prep(sample_time: np.ndarray, Cp: np.ndarray):
    """Build the AIF response matrix A, fit the J-term exponential basis, and
    pack the block lhsT / broadcast-ones / scale constants."""
    t_end = float(np.asarray(sample_time)[-1])
    Lf = int(round(t_end / STEP)) + 1
    t_samp = np.arange(Lf, dtype=np.float32) * np.float32(STEP)
    aifci = np.interp(
        t_samp.astype(np.float64),
        np.asarray(sample_time, np.float64),
        np.asarray(Cp, np.float64),
    ).astype(np.float32)
    aif = np.concatenate([np.zeros(DELAY, np.float32), aifci[:-DELAY]])
    idx = np.searchsorted(t_samp, np.asarray(sample_time, np.float32), side="left")
    idx = np.minimum(idx, Lf - 1)

    # A[k, t] = aif[idx[k] - t] for t <= idx[k] (conv 'full' sampled at idx)
    A = np.zeros((TS, 640), np.float64)
    for k in range(TS):
        i = int(idx[k])
        A[k, : i + 1] = aif[i::-1]

    alphas = _alphas()
    tg = np.arange(640, dtype=np.float64)
    g = np.geomspace(0.02, 64.0, 4000)
    P = np.exp(-0.1 * np.outer(g, tg)) @ A.T          # [G, 50]
    M = np.exp(-np.outer(g, alphas))                  # [G, J]
    B = np.linalg.solve(M.T @ M + 1e-8 * np.eye(J), M.T @ P)  # [J, 50]
    B = B.astype(np.float32)

    # block lhsT [128, 100]: blocks (em h0, em h1, ep h0, ep h1) x 32 alphas
    # (rows tensors are (thm h0, thm h1, thp h0, thp h1) so that each rows
    # quantity ships in one [128,100] -> [2,6400] reshape DMA)
    b4 = np.zeros((128, 2 * TS), np.float32)
    b4[0:J, 0:TS] = B
    b4[J : 2 * J, TS : 2 * TS] = B
    b4[2 * J : 3 * J, 0:TS] = B
    b4[3 * J : 4 * J, TS : 2 * TS] = B

    ones4 = np.zeros((4, 128), np.float32)
    for r in range(4):
        ones4[r, r * J : (r + 1) * J] = 1.0

    sv = (-alphas[np.arange(128) % J]).reshape(128, 1).astype(np.float32)
    return b4, ones4, sv


def kernel(param: np.ndarray, sample_time: np.ndarray, Cp: np.ndarray) -> np.ndarray:
    from concourse.bass_utils import run_bass_kernel_spmd

    if "nc" not in _CACHE:
        _CACHE["nc"] = _build_bass()
    nc = _CACHE["nc"]

    b4, ones4, sv = _host_prep(sample_time, Cp)
    pflat = np.ascontiguousarray(np.asarray(param, np.float32).reshape(4, NPIX))
    in_maps = []
    for c in range(NCORES):
        in_maps.append(
            {
                "pmap": np.ascontiguousarray(pflat[:, c * SHARD : (c + 1) * SHARD]),
                "b4": b4,
                "ones4": ones4,
                "sv": sv,
            }
        )
    res = run_bass_kernel_spmd(
        nc,
        in_maps,
        core_ids=list(range(NCORES)),
        trace=bool(int(os.environ.get("DCE_TRACE", "0"))),
    )
    if res.exec_time_ns is not None:
        _CACHE["exec_time_ns"] = res.exec_time_ns
    # sig2 [100, 6400] per core: rows 0-49 half0 pixels, rows 50-99 half1
    parts = []
    for r in res.results:
        s2 = r["sig2"]
        parts.append(np.concatenate([s2[:TS, :], s2[TS:, :]], axis=1))
    out = np.concatenate(parts, axis=1)
    return out.reshape(TS, 1, H, W)


# revision 12
# speedup vs baseline: 4.4059x; 1.0232x over previous
"""Trainium2 Bass kernel for the DCE (dynamic contrast-enhanced) 2CXM signal model.

Algorithmic core (replaces the 640-step FFT convolution of the reference):

  The sampled convolution response is, per pixel, p_k(theta) =
  sum_t A[k,t] e^{-0.1 t theta} evaluated at theta_m / theta_p -- a Laplace-
  type function of a single scalar.  It is approximated to ~1e-6 relative
  (vs a 2e-2 tolerance) by a J=32 sum of exponentials
      p_k(theta) ~= sum_j B[k,j] e^{-alpha_j theta}
  with alpha_0 = 0 and alpha_1..31 geometrically spaced; B is fitted on the
  host by ridge least squares over theta in [0.02, 64] (the attainable range
  for param in [0.05, 1]^4 is well inside).  conc = c1*p(theta_m) +
  c2*p(theta_p) with the same per-pixel c1/c2 closed forms as before.

  The SPGR epilogue uses the exact identity
      1/(1 - c e^{-u}) = (coth(u/2) + 1)/2,  u = TR*(R1 + R1CA*conc) - ln c
  so sig = (K2/2)/tanh(u/2) + (K1 + K2/2): one Tanh, one reciprocal, one
  affine.  Exp and Tanh share one ACT table set (no in-loop table loads).

Device layout (per core, 12800 pixels):
  - prep in pixel-partition layout [128, 100] (pixel = p*100 + q), with the
    thm|thp and c1|c2 quantities packed as column-halves of [128, 200] tiles
    so most chain ops run once at free-size 200; ~25 ops total.  theta rows
    ship to a rows4 [4, 6400] SBUF tensor (2 reshape DMAs), c rows (fp16) to
    DRAM for broadcast.
  - main loop over 7 pixel pairs (6 x 1024 + 256), 4-way stacked tiles
    [128 = 4 blocks x 32 alphas, Wp]: blocks (em h0, em h1, ep h0, ep h1).
    PE ones-matmul broadcasts theta rows -> PSUM, ACT computes the exp basis
    in ONE fp16 instruction per pair (per-partition scale = -alpha), a DMA
    broadcasts fp16 c rows, DVE multiplies at 2x fp16 throughput, and ONE
    fp16 matmul against the block lhsT B4 [128, 100] contracts basis -> conc
    for both halves at once ([100, Wp] PSUM: rows 0-49 half0, 50-99 half1).
  - epilogue: ACT Tanh, DVE reciprocal, Pool affine -> fp16, DMA out per pair.
"""

import os

import numpy as np

H = W = 320
NPIX = H * W
NCORES = 8
SHARD = NPIX // NCORES      # 12800 pixels per core
HALF = SHARD // 2           # 6400 (stacking half)
QP = 100                    # free size of the [128, 100] prep layout
PW = 1024                   # pixels per half per pair (2 PSUM banks)
NP = (HALF + PW - 1) // PW  # 7 pairs (6 x 1024 + 1 x 256)
J = 32                      # exponential-basis size
L = 589                     # fine time-grid length
TS = 50                     # output time samples
STEP = 0.1
DELAY = 30                  # 3s bolus delay in fine-grid samples

# SPGR constants (from reference.py)
SIG_BASELINE = 100.0
R1 = 1.0
R1CA = 4.3
FA = 10.0
TR = 0.00487

_CACHE: dict = {}


def _spgr_consts():
    fa = FA * np.pi / 180.0
    cosf = float(np.cos(np.float32(fa)))
    sinf = float(np.sin(np.float32(fa)))
    E1 = float(np.exp(np.float32(-TR * R1)))
    M0 = SIG_BASELINE * (1.0 - cosf * E1) / (sinf * (1.0 - E1))
    M0t = M0 * sinf
    M_steady = M0t * (1.0 - E1) / (1.0 - E1 * cosf)
    C0 = SIG_BASELINE - M_steady
    K1 = M0t / cosf + C0
    K2 = -M0t * (1.0 - cosf) / cosf
    a = TR * R1CA
    b = TR * R1 - float(np.log(cosf))
    return K1, K2, a, b


def _alphas():
    return np.concatenate(
        [[0.0], np.geomspace(0.05, 58.8, J - 1)]
    ).astype(np.float64)


def _build_bass():
    import concourse.bass as bass
    import concourse.tile as tile
    from concourse import bacc, mybir
    from contextlib import ExitStack

    f32 = mybir.dt.float32
    f32r = mybir.dt.float32r
    f16 = mybir.dt.float16
    AF = mybir.ActivationFunctionType
    ALU = mybir.AluOpType

    K1, K2, a_, b_ = _spgr_consts()

    nc = bacc.Bacc()
    pmap = nc.dram_tensor("pmap", [4, SHARD], f32, kind="ExternalInput")
    b4d = nc.dram_tensor("b4", [128, 2 * TS], f16, kind="ExternalInput")
    ones4d = nc.dram_tensor("ones4", [4, 128], f32, kind="ExternalInput")
    svd = nc.dram_tensor("sv", [128, 1], f32, kind="ExternalInput")
    sig2 = nc.dram_tensor("sig2", [2 * TS, HALF], f16, kind="ExternalOutput")

    with tile.TileContext(nc) as tc, ExitStack() as ctx:
        const = ctx.enter_context(tc.tile_pool(name="const", bufs=1))
        rows = ctx.enter_context(tc.tile_pool(name="rows", bufs=1))
        ebp = ctx.enter_context(tc.tile_pool(name="ebp", bufs=3))
        cbp = ctx.enter_context(tc.tile_pool(name="cbp", bufs=3))
        rhp = ctx.enter_context(tc.tile_pool(name="rhp", bufs=3))
        ep1 = ctx.enter_context(tc.tile_pool(name="ep1", bufs=2))
        ep2 = ctx.enter_context(tc.tile_pool(name="ep2", bufs=2))
        obp = ctx.enter_context(tc.tile_pool(name="obp", bufs=3))
        psbc = ctx.enter_context(
            tc.tile_pool(name="psbc", bufs=2, space=bass.MemorySpace.PSUM)
        )
        pcc = ctx.enter_context(
            tc.tile_pool(name="pcc", bufs=2, space=bass.MemorySpace.PSUM)
        )
        prep = ctx.enter_context(tc.tile_pool(name="prep", bufs=1))
        dpool = ctx.enter_context(tc.tile_pool(name="drows", bufs=1, space="DRAM"))

        V = nc.vector
        G = nc.gpsimd

        # ---- constants ----
        b4_sb = const.tile([128, 2 * TS], f16, tag="b4_sb", name="b4_sb")
        o4_sb = const.tile([4, 128], f32, tag="o4_sb", name="o4_sb")
        o4_r = const.tile([4, 128], f32r, tag="o4_r", name="o4_r")
        sv_sb = const.tile([128, 1], f32, tag="sv_sb", name="sv_sb")
        tb_sb = const.tile([2 * TS, 1], f32, tag="tb_sb", name="tb_sb")
        nc.scalar.dma_start(out=b4_sb[:], in_=b4d[:])
        nc.scalar.dma_start(out=o4_sb[:], in_=ones4d[:])
        nc.scalar.dma_start(out=sv_sb[:], in_=svd[:])
        G.tensor_copy(o4_r[:], o4_sb[:])
        G.memset(tb_sb, float(b_ / 2.0))

        # ---- per-pixel prep, [128, 100] pixel-partition layout ----
        def ptile(tag, w=QP):
            return prep.tile([128, w], f32, tag=tag, name=tag)

        pin1 = prep.tile([128, 2, QP], f32, tag="pin1", name="pin1")
        pin2 = prep.tile([128, 2, QP], f32, tag="pin2", name="pin2")
        # pin1 = (fp, ps), pin2 = (ve, vp): chain can start after pin1 lands
        nc.sync.dma_start(
            out=pin1, in_=pmap[2:4, :].rearrange("v (p q) -> p v q", p=128)
        )
        nc.sync.dma_start(
            out=pin2, in_=pmap[0:2, :].rearrange("v (p q) -> p v q", p=128)
        )
        fp = pin1[:, 0, :]; ps = pin1[:, 1, :]
        ve = pin2[:, 0, :]; vp = pin2[:, 1, :]
        pin1f = pin1.rearrange("p v q -> p (v q)")

        thmthp = ptile("thmthp", 2 * QP)
        c1c2 = prep.tile([128, 2 * QP], f16, tag="c1c2", name="c1c2")
        thm = thmthp[:, 0:QP]
        thp = thmthp[:, QP : 2 * QP]

        # critical chain (DVE unless noted)
        rfps = ptile("rfps", 2 * QP)
        V.reciprocal_approx_fast(rfps, pin1f)              # (1/fp, 1/ps)
        rfp = rfps[:, 0:QP]; rps = rfps[:, QP : 2 * QP]
        Te = ptile("Te"); V.tensor_mul(Te, ve, rps)
        sv2 = ptile("sv2"); V.tensor_add(sv2, vp, ve)
        T_ = ptile("T_"); V.tensor_mul(T_, sv2, rfp)
        s_ = ptile("s_"); V.tensor_add(s_, T_, Te)
        Tc = ptile("Tc"); G.tensor_mul(Tc, vp, rfp)        # Pool, off-chain
        q4 = ptile("q4")
        V.scalar_tensor_tensor(q4, Tc, 4.0, Te, op0=ALU.mult, op1=ALU.mult)
        sq = ptile("sq"); V.tensor_mul(sq, s_, s_)
        V.tensor_sub(sq, sq, q4)
        d_ = ptile("d_"); nc.scalar.sqrt(d_, sq)           # ACT (sqrt table)
        denq = ptile("denq", 2 * QP)
        den = denq[:, 0:QP]
        V.tensor_add(den, s_, d_)
        G.tensor_copy(denq[:, QP : 2 * QP], q4)            # Pool, off-chain
        rr1 = ptile("rr1", 2 * QP)
        V.reciprocal_approx_fast(rr1, denq)                # (1/den, 1/q4)
        V.tensor_scalar_mul(thm, rr1[:, 0:QP], 2.0)
        V.scalar_tensor_tensor(
            thp, den, 2.0, rr1[:, QP : 2 * QP], op0=ALU.mult, op1=ALU.mult
        )

        # theta rows ship as soon as thm/thp are written
        rowsT = rows.tile([4, HALF], f32, tag="rowsT", name="rowsT")
        nc.sync.dma_start(out=rowsT[0:2, :], in_=thmthp[:, 0:QP])
        nc.sync.dma_start(out=rowsT[2:4, :], in_=thmthp[:, QP : 2 * QP])

        # geometric-sum normalizers Sm/Sp, packed at free-size 200
        e1 = ptile("e1", 2 * QP)
        nc.scalar.activation(e1, thmthp, AF.Exp, bias=0.0, scale=-STEP)
        eL = ptile("eL", 2 * QP)
        nc.scalar.activation(eL, thmthp, AF.Exp, bias=0.0, scale=-STEP * L)
        Te2 = ptile("Te2", 2 * QP)                          # (Te, Te), off-chain
        G.tensor_copy(Te2[:, 0:QP], Te)
        G.tensor_copy(Te2[:, QP : 2 * QP], Te)
        umup = ptile("umup", 2 * QP)
        V.tensor_mul(umup, Te2, thmthp)                     # (Te*thm, Te*thp)
        albe = ptile("albe", 2 * QP)
        G.tensor_scalar(albe[:, 0:QP], umup[:, 0:QP], -1.0, 1.0,
                        op0=ALU.mult, op1=ALU.add)          # alp = 1 - Te*thm
        G.tensor_scalar_sub(albe[:, QP : 2 * QP], umup[:, QP : 2 * QP], 1.0)
        den1 = ptile("den1", 2 * QP)
        V.tensor_scalar(den1, e1, -1.0, 1.0, op0=ALU.mult, op1=ALU.add)
        V.reciprocal_approx_fast(den1, den1)                # 1/(1-e1)
        numL = ptile("numL", 2 * QP)
        G.tensor_scalar(numL, eL, -1.0, 1.0, op0=ALU.mult, op1=ALU.add)
        SmSp = ptile("SmSp", 2 * QP)
        V.tensor_mul(SmSp, numL, den1)                      # (Sm, Sp)
        Sm = SmSp[:, 0:QP]; Sp = SmSp[:, QP : 2 * QP]
        asbs = ptile("asbs", 2 * QP)
        V.tensor_mul(asbs, albe, SmSp)                      # (alp*Sm, bet*Sp)
        dsab = ptile("dsab", 2 * QP)
        V.tensor_sub(dsab[:, 0:QP], Sm, Sp)
        V.tensor_add(dsab[:, QP : 2 * QP], asbs[:, 0:QP], asbs[:, QP : 2 * QP])
        rr2 = ptile("rr2", 2 * QP)
        V.reciprocal_approx_fast(rr2, dsab)                 # (1/dS, 1/ab)
        rdS = rr2[:, 0:QP]; rab = rr2[:, QP : 2 * QP]
        w1 = ptile("w1"); G.tensor_mul(w1, vp, albe[:, 0:QP])     # off-chain
        w2 = ptile("w2"); G.tensor_mul(w2, vp, albe[:, QP : 2 * QP])
        vede = ptile("vede"); V.tensor_mul(vede, ve, rdS)
        w1b = ptile("w1b"); V.tensor_mul(w1b, w1, rab)
        w2b = ptile("w2b"); V.tensor_mul(w2b, w2, rab)
        V.tensor_add(c1c2[:, 0:QP], w1b, vede)              # c1 (fp16)
        V.tensor_sub(c1c2[:, QP : 2 * QP], w2b, vede)       # c2 (fp16)

        # c rows to DRAM (fp16) for per-pair broadcast DMAs
        crows_d = dpool.tile([4, HALF], f16, tag="crows_d", name="crows_d")
        nc.scalar.dma_start(out=crows_d[0:2, :], in_=c1c2[:, 0:QP])
        nc.scalar.dma_start(out=crows_d[2:4, :], in_=c1c2[:, QP : 2 * QP])

        # ---- main loop over pixel pairs ----
        tanh_scale = float(a_ / 2.0)
        fs1 = float(K2 / 2.0)
        fs2 = float(K1 + K2 / 2.0)

        for p in range(NP):
            lo = p * PW
            Wp = min(PW, HALF - lo)
            sl = slice(lo, lo + Wp)

            th_bc = psbc.tile([128, PW], f32, tag="th_bc", name="th_bc")
            for mlo in range(0, Wp, 512):
                mw = min(512, Wp - mlo)
                nc.tensor.matmul(
                    th_bc[:, mlo : mlo + mw], o4_r[:],
                    rowsT[:, lo + mlo : lo + mlo + mw].bitcast(f32r),
                    start=True, stop=True,
                )
            eb = ebp.tile([128, PW], f16, tag="eb", name="eb")
            nc.scalar.activation(
                eb[:, :Wp], th_bc[:, :Wp], AF.Exp, bias=0.0, scale=sv_sb[:, 0:1]
            )
            cb = cbp.tile([128, PW], f16, tag="cb", name="cb")
            c0 = crows_d[0, lo : lo + Wp]
            nc.sync.dma_start(
                out=cb[:, :Wp],
                in_=bass.AP(
                    tensor=c0.tensor, offset=c0.offset,
                    ap=[[HALF, 4], [0, 32], [1, Wp]],
                ),
            )
            rhs = rhp.tile([128, PW], f16, tag="rhs", name="rhs")
            V.tensor_mul(rhs[:, :Wp], eb[:, :Wp], cb[:, :Wp])

            conc = pcc.tile([2 * TS, PW], f32, tag="conc", name="conc")
            for mlo in range(0, Wp, 512):
                mw = min(512, Wp - mlo)
                nc.tensor.matmul(
                    conc[:, mlo : mlo + mw], b4_sb[:],
                    rhs[:, mlo : mlo + mw], start=True, stop=True,
                )

            tht = ep1.tile([2 * TS, PW], f32, tag="tht", name="tht")
            nc.scalar.activation(
                tht[:, :Wp], conc[:, :Wp], AF.Tanh,
                bias=tb_sb, scale=tanh_scale,
            )
            rt = ep2.tile([2 * TS, PW], f32, tag="rt", name="rt")
            V.reciprocal_approx_fast(rt[:, :Wp], tht[:, :Wp])
            ob = obp.tile([2 * TS, PW], f16, tag="ob", name="ob")
            G.tensor_scalar(
                ob[:, :Wp], rt[:, :Wp], fs1, fs2, op0=ALU.mult, op1=ALU.add
            )
            nc.sync.dma_start(out=sig2[:, sl], in_=ob[:, :Wp])

    nc.compile()
    return nc


def _host_prep(sample_time: np.ndarray, Cp: np.ndarray):
    """Build the AIF response matrix A, fit the J-term exponential basis, and
    pack the block lhsT / broadcast-ones / scale constants."""
    t_end = float(np.asarray(sample_time)[-1])
    Lf = int(round(t_end / STEP)) + 1
    t_samp = np.arange(Lf, dtype=np.float32) * np.float32(STEP)
    aifci = np.interp(
        t_samp.astype(np.float64),
        np.asarray(sample_time, np.float64),
        np.asarray(Cp, np.float64),
    ).astype(np.float32)
    aif = np.concatenate([np.zeros(DELAY, np.float32), aifci[:-DELAY]])
    idx = np.searchsorted(t_samp, np.asarray(sample_time, np.float32), side="left")
    idx = np.minimum(idx, Lf - 1)

    # A[k, t] = aif[idx[k] - t] for t <= idx[k] (conv 'full' sampled at idx)
    A = np.zeros((TS, 640), np.float64)
    for k in range(TS):
        i = int(idx[k])
        A[k, : i + 1] = aif[i::-1]

    alphas = _alphas()
    tg = np.arange(640, dtype=np.float64)
    g = np.geomspace(0.02, 64.0, 4000)
    P = np.exp(-0.1 * np.outer(g, tg)) @ A.T          # [G, 50]
    M = np.exp(-np.outer(g, alphas))                  # [G, J]
    B = np.linalg.solve(M.T @ M + 1e-8 * np.eye(J), M.T @ P)  # [J, 50]
    B = B.astype(np.float32)

    # block lhsT [128, 100]: blocks (em h0, em h1, ep h0, ep h1) x 32 alphas
    # (rows tensors are (thm h0, thm h1, thp h0, thp h1) so that each rows
    # quantity ships in one [128,100] -> [2,6400] reshape DMA)
    b4 = np.zeros((128, 2 * TS), np.float32)
    b4[0:J, 0:TS] = B
    b4[J : 2 * J, TS : 2 * TS] = B
    b4[2 * J : 3 * J, 0:TS] = B
    b4[3 * J : 4 * J, TS : 2 * TS] = B

    ones4 = np.zeros((4, 128), np.float32)
    for r in range(4):
        ones4[r, r * J : (r + 1) * J] = 1.0

    sv = (-alphas[np.arange(128) % J]).reshape(128, 1).astype(np.float32)
    return b4.astype(np.float16), ones4, sv


def kernel(param: np.ndarray, sample_time: np.ndarray, Cp: np.ndarray) -> np.ndarray:
    from concourse.bass_utils import run_bass_kernel_spmd

    if "nc" not in _CACHE:
        _CACHE["nc"] = _build_bass()
    nc = _CACHE["nc"]

    b4, ones4, sv = _host_prep(sample_time, Cp)
    pflat = np.ascontiguousarray(np.asarray(param, np.float32).reshape(4, NPIX))
    in_maps = []
    for c in range(NCORES):
        in_maps.append(
            {
                "pmap": np.ascontiguousarray(pflat[:, c * SHARD : (c + 1) * SHARD]),
                "b4": b4,
                "ones4": ones4,
                "sv": sv,
            }
        )
    res = run_bass_kernel_spmd(
        nc,
        in_maps,
        core_ids=list(range(NCORES)),
        trace=bool(int(os.environ.get("DCE_TRACE", "0"))),
    )
    if res.exec_time_ns is not None:
        _CACHE["exec_time_ns"] = res.exec_time_ns
    # sig2 [100, 6400] fp16 per core: rows 0-49 half0 pixels, rows 50-99 half1
    parts = []
    for r in res.results:
        s2 = np.asarray(r["sig2"], np.float32)
        parts.append(np.concatenate([s2[:TS, :], s2[TS:, :]], axis=1))
    out = np.concatenate(parts, axis=1)
    return out.reshape(TS, 1, H, W)
